# revision 9
# baseline (speedup 1.0000x reference)
"""Delay-and-sum (DAS) beamforming kernel for 8 Trainium2 NeuronCores.

Problem: out[b,p] = sum_d apod[d] * lerp(S[b,d], tof[p,d]) / sum(apod)
  with S = sino[b,0,d,:], lerp via floor index k0 and fraction alpha.

Sharding: data-parallel over pixels (8192 pixels per core); no collectives.

Per-core pipeline (gather done ON-CHIP via GPSIMD ap_gather):
  - sino relaid host-side as sgp[d, 4t+b] (batch-minor words, f32, padded).
  - 16 detector rounds of 8 detectors (one per Q7 core). Data tile D:
    partition 16c+j holds sgp[8g+c, j%8 : j%8+8192] (word-shifted copies,
    built per round by 16 partition-strided DMAs), so a single shared index
    4*k0 per (pixel, core) gathers tap t / batch b at partition shift
    j = 4t+b (j in [0,8); j in [8,16) duplicates, masked off by wsel zeros).
  - one ap_gather per round: G[16c+j, i] = D[16c+j, idx_i] (8192 indices).
  - Interp weights ship as uint8 (q_t = round(255*w_t*valid)); the device
    rebuilds A_t = apod[d]/norm * w_t * valid via a copy + per-partition
    scale, spreads A across partitions with a PE matmul (wsel), applies it
    on DVE (G *= W read from PSUM), and reduces over (detector, tap)
    partitions keeping batch via a second PE matmul (red) + DVE accumulate.
"""
import numpy as np

import concourse.bass as bass
import concourse.tile as tile
from concourse import bacc, mybir

N_DET, N_T, NY, NX, B = 128, 2048, 256, 256, 4
P_TOTAL = NY * NX
N_CORES = 8
PX_PER_CORE = P_TOTAL // N_CORES          # 8192
N_ROUNDS = 16                             # detector rounds: 8 detectors each
SG_ROW = 4 * N_T + 16                     # 8208 padded words per detector
Q = 16                                    # 512-wide pieces per round
F32 = mybir.dt.float32
I16 = mybir.dt.int16
U8 = mybir.dt.uint8


def _build_kernel():
    nc = bacc.Bacc("TRN2", target_bir_lowering=False, debug=False)

    sgp = nc.dram_tensor("sgp", [N_DET, SG_ROW], F32, kind="ExternalInput")
    idxt = nc.dram_tensor("idxt", [N_ROUNDS * 128, PX_PER_CORE // 16], I16,
                          kind="ExternalInput")
    qt = nc.dram_tensor("qt", [N_ROUNDS * 16, PX_PER_CORE], U8,
                        kind="ExternalInput")
    apodt = nc.dram_tensor("apodt", [N_ROUNDS * 16, 1], F32,
                           kind="ExternalInput")
    red = nc.dram_tensor("red", [128, B], F32, kind="ExternalInput")
    wsel = nc.dram_tensor("wsel", [16, 128], F32, kind="ExternalInput")
    outd = nc.dram_tensor("out", [B, PX_PER_CORE], F32, kind="ExternalOutput")

    with tile.TileContext(nc) as tc:
        with (
            tc.tile_pool(name="const", bufs=1) as cpool,
            tc.tile_pool(name="dpool", bufs=2) as dpool,
            tc.tile_pool(name="qpool", bufs=1) as qpool,
            tc.tile_pool(name="apool", bufs=1) as apool,
            tc.tile_pool(name="ipool", bufs=2) as ipool,
            tc.tile_pool(name="gpool", bufs=2) as gpool,
            tc.tile_pool(name="cppool", bufs=2) as cppool,
            tc.tile_pool(name="opool", bufs=1) as opool,
            tc.tile_pool(name="wps", bufs=2, space="PSUM") as wps,
            tc.tile_pool(name="rps", bufs=2, space="PSUM") as rps,
        ):
            red_tl = cpool.tile([128, B], F32)
            nc.sync.dma_start(out=red_tl[:], in_=red.ap())
            wsel_tl = cpool.tile([16, 128], F32)
            nc.sync.dma_start(out=wsel_tl[:], in_=wsel.ap())

            acc = opool.tile([B, PX_PER_CORE], F32)

            for g in range(N_ROUNDS):
                D = dpool.tile([128, 4 * N_T], F32, tag="D")
                for j in range(16):
                    jj = j % 8
                    nc.sync.dma_start(
                        out=D[j:128:16, :],
                        in_=sgp.ap()[8 * g:8 * g + 8, jj:jj + 4 * N_T])
                apc = cppool.tile([16, 1], F32, tag="apc")
                nc.sync.dma_start(
                    out=apc[:], in_=apodt.ap()[g * 16:(g + 1) * 16, :])

                idx = ipool.tile([128, PX_PER_CORE // 16], I16, tag="idx")
                nc.sync.dma_start(
                    out=idx[:], in_=idxt.ap()[g * 128:(g + 1) * 128, :])
                qtl = qpool.tile([16, PX_PER_CORE], U8, tag="q")
                nc.sync.dma_start(
                    out=qtl[:], in_=qt.ap()[g * 16:(g + 1) * 16, :])
                A = apool.tile([16, PX_PER_CORE], F32, tag="A")
                nc.vector.tensor_copy(out=A[:], in_=qtl[:])
                nc.vector.tensor_scalar(
                    out=A[:], in0=A[:], scalar1=apc[:], scalar2=None,
                    op0=mybir.AluOpType.mult)

                G = gpool.tile([128, PX_PER_CORE], F32, tag="G")
                nc.gpsimd.ap_gather(
                    out_ap=G[:], in_ap=D[:], idxs_ap=idx[:],
                    channels=128, num_elems=4 * N_T, d=1,
                    num_idxs=PX_PER_CORE)

                for q in range(Q):
                    qs = slice(q * 512, (q + 1) * 512)
                    wp = wps.tile([128, 512], F32, tag="wp")
                    nc.tensor.matmul(out=wp[:], lhsT=wsel_tl[:],
                                     rhs=A[:, qs], start=True, stop=True)
                    nc.vector.tensor_tensor(
                        out=G[:, qs], in0=G[:, qs], in1=wp[:],
                        op=mybir.AluOpType.mult)
                    rp = rps.tile([B, 512], F32, tag="rp")
                    nc.tensor.matmul(out=rp[:], lhsT=red_tl[:],
                                     rhs=G[:, qs], start=True, stop=True)
                    if g == 0:
                        nc.vector.tensor_copy(out=acc[:, qs], in_=rp[:])
                    else:
                        nc.vector.tensor_tensor(
                            out=acc[:, qs], in0=acc[:, qs], in1=rp[:],
                            op=mybir.AluOpType.add)

            nc.sync.dma_start(out=outd.ap(), in_=acc[:])

    nc.compile()
    return nc


def _host_prep(sino: np.ndarray, lut: np.ndarray):
    sino = np.ascontiguousarray(sino, dtype=np.float32)
    lut = np.ascontiguousarray(lut, dtype=np.float32)

    sgp = np.zeros((N_DET, SG_ROW), dtype=np.float32)
    sgp[:, :4 * N_T] = sino[:, 0].transpose(1, 2, 0).reshape(N_DET, 4 * N_T)

    apod = (0.5 - 0.5 * np.cos(
        2.0 * np.pi * np.arange(N_DET, dtype=np.float32) / (N_DET - 1)
    )).astype(np.float32)
    norm = max(apod.sum(), np.finfo(np.float32).tiny)
    apod_n = (apod / norm).astype(np.float32)

    lut_flat = lut.reshape(P_TOTAL, N_DET, 2)
    tof = lut_flat[:, :, 0]
    alpha = lut_flat[:, :, 1]
    k_floor = np.floor(tof)
    valid = ((k_floor >= 0) & (k_floor < N_T - 1)).astype(np.float32)
    k0 = np.clip(k_floor, 0, N_T - 2).astype(np.int32)
    idx16 = (4 * k0).astype(np.int16)                       # [P, 128]
    q0 = np.rint(255.0 * (1.0 - alpha) * valid).astype(np.uint8)
    q1 = np.rint(255.0 * alpha * valid).astype(np.uint8)

    apodt = np.zeros((N_ROUNDS * 16, 1), dtype=np.float32)
    for g in range(N_ROUNDS):
        for t in range(2):
            for c in range(8):
                apodt[g * 16 + 8 * t + c, 0] = apod_n[8 * g + c] / 255.0

    red = np.zeros((128, B), dtype=np.float32)
    for c in range(8):
        for t in range(2):
            for b in range(B):
                red[16 * c + 4 * t + b, b] = 1.0
    wsel = np.zeros((16, 128), dtype=np.float32)
    for t in range(2):
        for c in range(8):
            for b in range(B):
                wsel[8 * t + c, 16 * c + 4 * t + b] = 1.0

    in_maps = []
    for core in range(N_CORES):
        pr = slice(core * PX_PER_CORE, (core + 1) * PX_PER_CORE)
        # [s, jp, g, c] -> [g, c, jp, s]
        ix = idx16[pr].reshape(PX_PER_CORE // 16, 16, N_ROUNDS, 8)
        ix = np.ascontiguousarray(ix.transpose(2, 3, 1, 0)).reshape(
            N_ROUNDS * 128, PX_PER_CORE // 16)
        # [t, i, g, c] -> [g, t, c, i]
        qq = np.stack([q0[pr], q1[pr]], axis=0)  # [t, P/core, 128]
        qq = qq.reshape(2, PX_PER_CORE, N_ROUNDS, 8)
        qq = np.ascontiguousarray(qq.transpose(2, 0, 3, 1)).reshape(
            N_ROUNDS * 16, PX_PER_CORE)
        in_maps.append({
            "sgp": sgp,
            "idxt": ix,
            "qt": qq,
            "apodt": apodt,
            "red": red,
            "wsel": wsel,
        })
    return in_maps


def _assemble(results: list) -> np.ndarray:
    outs = [r["out"] for r in results]                       # each [B, 8192]
    full = np.concatenate(outs, axis=1)                      # [B, P_TOTAL]
    return np.ascontiguousarray(full).reshape(B, 1, NY, NX)


_CACHE: dict = {}


def _get_nc():
    if "nc" not in _CACHE:
        _CACHE["nc"] = _build_kernel()
    return _CACHE["nc"]


def kernel(sino: np.ndarray, lut: np.ndarray) -> np.ndarray:
    from concourse.bass_utils import run_bass_kernel_spmd

    nc = _get_nc()
    in_maps = _host_prep(np.asarray(sino), np.asarray(lut))
    res = run_bass_kernel_spmd(nc, in_maps, core_ids=list(range(N_CORES)))
    return _assemble(res.results)


def kernel_timed(inputs: dict, iters: int = 20) -> float:
    """Run the kernel repeatedly with device-resident inputs; return ns/iter.

    The `iters` kernel executions run back-to-back inside a single jitted
    program (the bass primitive is effectful, so calls are not CSE'd), which
    keeps inputs device-resident and amortizes per-dispatch host overhead.
    """
    import time
    import jax
    from jax.sharding import Mesh, PartitionSpec
    from jax.experimental.shard_map import shard_map
    from concourse.bass2jax import (
        _bass_exec_p, install_neuronx_cc_hook)
    import concourse.mybir as mybir_

    nc = _get_nc()
    in_maps = _host_prep(np.asarray(inputs["sino"]), np.asarray(inputs["lut"]))

    install_neuronx_cc_hook()
    part_name = nc.partition_id_tensor.name if nc.partition_id_tensor else None
    in_names, out_names, out_avals, zero_outs = [], [], [], []
    for alloc in nc.m.functions[0].allocations:
        if not isinstance(alloc, mybir_.MemoryLocationSet):
            continue
        name = alloc.memorylocations[0].name
        if alloc.kind == "ExternalInput":
            if name != part_name:
                in_names.append(name)
        elif alloc.kind == "ExternalOutput":
            out_names.append(name)
            shape = tuple(alloc.tensor_shape)
            dtype = mybir_.dt.np(alloc.dtype)
            out_avals.append(jax.core.ShapedArray(shape, dtype))
            zero_outs.append(np.zeros(shape, dtype))
    n_params = len(in_names)
    all_names = in_names + out_names
    if part_name is not None:
        all_names.append(part_name)
    from concourse.bass2jax import partition_id_tensor

    def _body(*args):
        operands = list(args)
        if part_name is not None:
            operands.append(partition_id_tensor())
        outs = None
        for _ in range(iters):
            outs = _bass_exec_p.bind(
                *operands,
                out_avals=tuple(out_avals),
                in_names=tuple(all_names),
                out_names=tuple(out_names),
                lowering_input_output_aliases=(),
                sim_require_finite=True,
                sim_require_nnan=True,
                nc=nc,
            )
        return tuple(outs)

    devices = jax.devices()[:N_CORES]
    mesh = Mesh(np.asarray(devices), ("core",))
    n_outs = len(out_names)
    sharded = jax.jit(
        shard_map(_body, mesh=mesh,
                  in_specs=(PartitionSpec("core"),) * (n_params + n_outs),
                  out_specs=(PartitionSpec("core"),) * n_outs,
                  check_rep=False),
        keep_unused=True,
    )
    concat_in = [
        np.concatenate([in_maps[c][name] for c in range(N_CORES)], axis=0)
        for name in in_names
    ]
    concat_zeros = [
        np.zeros((N_CORES * z.shape[0], *z.shape[1:]), z.dtype) for z in zero_outs
    ]
    dev_in = [jax.device_put(a) for a in concat_in]
    dev_zero = [jax.device_put(a) for a in concat_zeros]

    # warmup (compile + 2 runs)
    for _ in range(3):
        outs = sharded(*dev_in, *dev_zero)
        jax.block_until_ready(outs)

    n_calls = 3
    t0 = time.perf_counter()
    for _ in range(n_calls):
        outs = sharded(*dev_in, *dev_zero)
    jax.block_until_ready(outs)
    t1 = time.perf_counter()
    return (t1 - t0) / (n_calls * iters) * 1e9
